# revision 22
# baseline (speedup 1.0000x reference)
"""ConditionalRandomField loss kernel for Trainium2 (8 NeuronCores).

Math (per sequence b):
    loss[b] = log_score(gold path) - log_partition
The log_partition forward recursion is computed in exp space:
    f_t[j] = (sum_i E[i,j] * f_{t-1}[i]) * g_t[j]
with E = exp(transitions), g_t = exp(emissions_t), f_0 = exp(start)*g_0,
and periodic per-batch rescaling whose log is accumulated separately:
    log_partition = log(sum_j f_{L-1}[j]*exp(stop[j])) + sum_m log(s_m).

Sharding: data-parallel over batch; core c owns sequences [8c, 8c+8).
Per core the 256-tag state is held as a [128 x (2 jchunk x 8 batch)] bf16
tile; each scan step is 4 PE matmuls (E tiles stationary, bf16 FWL) and one
VE multiply (psum * exp(emissions)).  The gold-path numerator uses
one-hot tag masks: emissions[b,t,tag] via tensor_tensor_reduce against the
staged emission chunks, transitions[tag_t,tag_t+1] via y = Tr^T @ OH matmuls
followed by tensor_tensor_reduce against the shifted one-hot, start/stop via
tiny matmuls.  (HW indirect-DMA only gathers one row per partition, so
per-element gathers are done with masks instead.)

NOTE: mask is all-ones for this problem spec (fill: ones); the kernel
assumes it (the reference's masked branches are identities then).
"""

import numpy as np
from contextlib import ExitStack

import concourse.bass as bass
import concourse.bacc as bacc
import concourse.tile as tile
from concourse import mybir
from concourse.bass_utils import run_bass_kernel_spmd

F32 = mybir.dt.float32
BF16 = mybir.dt.bfloat16
I32 = mybir.dt.int32

NCORES = 8
B = 64
L = 1024
T = 256
BC = B // NCORES      # sequences per core
PJ = 128              # partition tile of the tag dim
JCN = T // PJ         # = 2 tag chunks
RS = 16               # rescale sampling period (steps)
DEFER = 2             # rescale applied this many steps after sampling
TCH = 128             # emission-load chunk (timesteps per DMA/exp chunk)
CSH = 6.5             # constant log-shift folded into E = exp(transitions - CSH)

AUX_START = T * T          # aux table: [transitions-CSH | start | stop | transitions]
AUX_STOP = T * T + T
AUX_TRRAW = T * T + 2 * T
AUX_N = 2 * T * T + 2 * T


def _sample_steps(length):
    return [t for t in range(1, length) if t % RS == 0 and t + DEFER <= length - 1]


def build_program(length=L, use_gpsimd_oh=False, do_emis=True, do_trans=True, do_ssmm=True):
    """Build the single-core SPMD bass program (each core runs the same
    program on its own batch shard)."""
    assert length % 16 == 0
    l16 = length // 16
    nsamp = len(_sample_steps(length))
    nspad = max(8, ((nsamp + 7) // 8) * 8)

    nc = bacc.Bacc()
    em_t = nc.declare_dram_parameter("em", [BC * length * T, 1], F32, isOutput=False)
    aux_t = nc.declare_dram_parameter("aux", [AUX_N, 1], F32, isOutput=False)
    tags_t = nc.declare_dram_parameter("tags_tb", [length * BC, 1], F32, isOutput=False)
    iota_t = nc.declare_dram_parameter("iota", [128, 1], F32, isOutput=False)
    loss_t = nc.declare_dram_parameter("loss", [BC, 1], F32, isOutput=True)

    def dram_ap(handle, offset, ap):
        full = handle[:]
        return bass.AP(tensor=full.tensor, offset=offset, ap=ap)

    with tile.TileContext(nc) as tc, ExitStack() as ctx:
        const = ctx.enter_context(tc.tile_pool(name="const", bufs=1))
        stage = ctx.enter_context(tc.tile_pool(name="stage", bufs=2))
        gpool = ctx.enter_context(tc.tile_pool(name="gpool", bufs=1))
        fpool = ctx.enter_context(tc.tile_pool(name="fpool", bufs=3))
        vpool = ctx.enter_context(tc.tile_pool(name="vpool", bufs=2))
        ppool = ctx.enter_context(tc.tile_pool(name="ppool", bufs=2, space="PSUM"))
        spool = ctx.enter_context(tc.tile_pool(name="spool", bufs=1, space="PSUM"))
        smallp = ctx.enter_context(tc.tile_pool(name="smallp", bufs=2, space="PSUM"))

        # ---------------- constants / setup ----------------
        # E = exp(transitions) as two [128, 256] bf16 tiles (i-chunk major).
        e_tiles = []
        tr_tiles = []
        for ic in range(JCN):
            eraw = stage.tile([128, T], F32, name=f"eraw{ic}", tag="eraw")
            nc.sync.dma_start(
                out=eraw,
                in_=dram_ap(aux_t, ic * 128 * T, [[T, 128], [1, T]]),
            )
            ebf = const.tile([128, T], BF16, name=f"ebf{ic}")
            # NOTE: the host uploads transitions - CSH in aux, so E here is
            # exp(transitions - CSH): per-step growth ~e^0 keeps running sums
            # inside the ACT Ln range.  The same shift flows into the
            # numerator's transition gathers, so it cancels in the loss.
            nc.scalar.activation(
                out=ebf, in_=eraw, func=mybir.ActivationFunctionType.Exp
            )
            e_tiles.append(ebf)
            # unshifted transitions in bf16 for the gold-score matmuls
            eraw2 = stage.tile([128, T], F32, name=f"eraw2_{ic}", tag="eraw")
            nc.sync.dma_start(
                out=eraw2,
                in_=dram_ap(aux_t, AUX_TRRAW + ic * 128 * T, [[T, 128], [1, T]]),
            )
            trbf = const.tile([128, T], BF16, name=f"trbf{ic}")
            nc.vector.tensor_copy(out=trbf, in_=eraw2)
            tr_tiles.append(trbf)

        # exp(start) [128, 2] f32; exp(stop) [128, 2] bf16
        ssraw = stage.tile([128, 2 * JCN], F32, name="ssraw", tag="eraw")
        nc.sync.dma_start(
            out=ssraw[:, 0:JCN],
            in_=dram_ap(aux_t, AUX_START, [[1, 128], [128, JCN]]),
        )
        nc.sync.dma_start(
            out=ssraw[:, JCN : 2 * JCN],
            in_=dram_ap(aux_t, AUX_STOP, [[1, 128], [128, JCN]]),
        )
        sstart = const.tile([128, JCN], F32, name="sstart")
        nc.scalar.activation(
            out=sstart, in_=ssraw[:, 0:JCN], func=mybir.ActivationFunctionType.Exp
        )
        sstop = const.tile([128, JCN], BF16, name="sstop")
        nc.scalar.activation(
            out=sstop, in_=ssraw[:, JCN : 2 * JCN], func=mybir.ActivationFunctionType.Exp
        )
        ssbf = const.tile([128, 2 * JCN], BF16, name="ssbf")
        nc.vector.tensor_copy(out=ssbf, in_=ssraw)

        ones_w = const.tile([128, 128], BF16, name="ones_w")
        nc.vector.memset(ones_w, 1.0)
        ones_col = const.tile([128, 1], BF16, name="ones_col")
        nc.vector.memset(ones_col, 1.0)

        logsbuf = const.tile([BC, nspad], F32, name="logsbuf")
        nc.vector.memset(logsbuf, 1.0)  # log(1)=0 padding

        # ---------------- numerator: one-hot masks ----------------
        # OH_jc[p, t*BC + b] = 1.0 iff tags[b, t] == jc*128 + p, bf16,
        # with BC zero columns of padding at t == length (for the t+1 shift).
        iota_sb = const.tile([128, 1], F32, name="iota_sb")
        nc.sync.dma_start(out=iota_sb, in_=iota_t[:])
        tags_bc = stage.tile([128, length * BC], F32, name="tags_bc", tag="tags_bc")
        nc.sync.dma_start(
            out=tags_bc,
            in_=dram_ap(tags_t, 0, [[0, 128], [1, length * BC]]),
        )
        noh = (length + 1) * BC
        oh_tiles = []
        for jc in range(JCN):
            oh = const.tile([128, noh], BF16, name=f"oh{jc}")
            oh_engine = nc.gpsimd if use_gpsimd_oh else nc.vector
            oh_engine.tensor_scalar(
                out=oh[:, 0 : length * BC],
                in0=tags_bc,
                scalar1=float(jc * 128),
                scalar2=iota_sb[:],
                op0=mybir.AluOpType.subtract,
                op1=mybir.AluOpType.is_equal,
            )
            nc.vector.memset(oh[:, length * BC : noh], 0.0)
            oh_tiles.append(oh)

        # per-(tag-partition, b) accumulators for emission+transition scores:
        # each fused multiply-accumulate call writes its partial sum into a
        # distinct column; folded at the end with two strided reduces.
        tch = min(TCH, length)
        tblk = min(512, length)
        ne_calls = (length // tch) * JCN
        nt_calls = (length // tblk) * JCN
        acc2e = const.tile([128, ne_calls * BC], F32, name="acc2e")
        acc2t = const.tile([128, nt_calls * BC], F32, name="acc2t")
        scr = const.tile([128, 512], BF16, name="scr")

        # ---------------- emissions -> g = exp(emissions), bf16 ----------------
        gbuf = gpool.tile([128, length, JCN, BC], BF16, name="gbuf")
        for tci in range(length // tch):
            raw = stage.tile([128, tch, JCN, BC], F32, name="raw", tag="raw")
            # em is host-pretransposed to [p, t, jc, b]: fully contiguous load
            row = length * JCN * BC
            nc.sync.dma_start(
                out=raw,
                in_=dram_ap(
                    em_t,
                    tci * tch * JCN * BC,
                    [[row, 128], [1, tch * JCN * BC]],
                ),
            )
            nc.scalar.activation(
                out=gbuf[:, tci * tch : (tci + 1) * tch, :, :],
                in_=raw,
                func=mybir.ActivationFunctionType.Exp,
            )
            # emission part of the gold score: sum_t raw[p,(t,jc,b)]*OH[p,t,b]
            for jc in range(JCN if do_emis else 0):
                for b in range(BC):
                    c0 = tci * tch * BC + b
                    acol = (tci * JCN + jc) * BC + b
                    nc.vector.scalar_tensor_tensor(
                        out=scr[:, 0:tch],
                        in0=raw[:, :, jc, b],
                        scalar=1.0,
                        in1=oh_tiles[jc][:, c0 : c0 + (tch - 1) * BC + 1 : BC],
                        op0=mybir.AluOpType.mult,
                        op1=mybir.AluOpType.mult,
                        accum_out=acc2e[:, acol : acol + 1],
                    )

        # ---------------- the scan: NCH independent batch-chains ----------------
        # Each chain owns BCH sequences; chains interleave on the PE so the
        # per-step latency chain (mm drain -> VE multiply -> next mm) of one
        # chain hides inside the other's.
        NCH = 2
        BCH = BC // NCH
        bsl = [slice(ch * BCH, (ch + 1) * BCH) for ch in range(NCH)]

        fs = []
        for ch in range(NCH):
            f = fpool.tile([128, JCN, BCH], BF16, name=f"f{ch}", tag=f"f{ch}")
            for jc in range(JCN):
                nc.vector.tensor_scalar_mul(
                    out=f[:, jc, :],
                    in0=gbuf[:, 0, jc, bsl[ch]],
                    scalar1=sstart[:, jc : jc + 1],
                )
            fs.append(f)

        logsbufs = []
        for ch in range(NCH):
            lsb = const.tile([BCH, nspad], F32, name=f"logsbuf{ch}")
            nc.vector.memset(lsb, 1.0)
            logsbufs.append(lsb)

        msamp = 0
        for t in range(1, length):
            sample = t % RS == 0 and t + DEFER <= length - 1
            for ch in range(NCH):
                f = fs[ch]
                p = ppool.tile([128, JCN, BCH], F32, name=f"p{ch}", tag=f"p{ch}")
                for jc in range(JCN):
                    nc.tensor.matmul(
                        out=p[:, jc, :],
                        lhsT=e_tiles[0][:, jc * 128 : (jc + 1) * 128],
                        rhs=f[:, 0, :],
                        start=True,
                        stop=False,
                    )
                    nc.tensor.matmul(
                        out=p[:, jc, :],
                        lhsT=e_tiles[1][:, jc * 128 : (jc + 1) * 128],
                        rhs=f[:, 1, :],
                        start=False,
                        stop=True,
                    )
                fn = fpool.tile([128, JCN, BCH], BF16, name=f"f{ch}", tag=f"f{ch}")
                nc.vector.tensor_tensor(
                    out=fn[:],
                    in0=p[:],
                    in1=gbuf[:, t, :, bsl[ch]],
                    op=mybir.AluOpType.mult,
                )
                fs[ch] = fn

                if sample:
                    f = fn
                    s_bc = spool.tile([128, BCH], F32, name="s_bc", tag="s")
                    nc.tensor.matmul(
                        out=s_bc, lhsT=ones_w, rhs=f[:, 0, :], start=True, stop=False
                    )
                    nc.tensor.matmul(
                        out=s_bc, lhsT=ones_w, rhs=f[:, 1, :], start=False, stop=True
                    )
                    s4 = smallp.tile([BCH, 1], F32, name="s4", tag="small")
                    nc.tensor.matmul(
                        out=s4, lhsT=f[:, 0, :], rhs=ones_col, start=True, stop=False
                    )
                    nc.tensor.matmul(
                        out=s4, lhsT=f[:, 1, :], rhs=ones_col, start=False, stop=True
                    )
                    v = vpool.tile([128, BCH], F32, name="v", tag="v")
                    nc.vector.reciprocal(out=v, in_=s_bc)
                    nc.vector.tensor_copy(
                        out=logsbufs[ch][:, msamp : msamp + 1], in_=s4
                    )
                    for jc in range(JCN):
                        nc.vector.tensor_mul(
                            out=gbuf[:, t + DEFER, jc, bsl[ch]],
                            in0=gbuf[:, t + DEFER, jc, bsl[ch]],
                            in1=v,
                        )
            if sample:
                msamp += 1
        assert msamp == nsamp

        # ---------------- transition part of the gold score ----------------
        # y[j',(t,b)] = sum_i Tr[i,j'] * OH_t[i,(t,b)]; then
        # sum_t y[j',(t,b)] * OH_{t+1}[j',(t,b)] accumulated into numacc.
        for b in range(BC if do_trans else 0):
            for tc2 in range(length // tblk):
                for jcp in range(JCN):
                    y_ps = ppool.tile([128, tblk], F32, name="y_ps", tag="p0")
                    c0 = tc2 * tblk * BC + b
                    for ic in range(JCN):
                        nc.tensor.matmul(
                            out=y_ps,
                            lhsT=tr_tiles[ic][:, jcp * 128 : (jcp + 1) * 128],
                            rhs=oh_tiles[ic][:, c0 : c0 + (tblk - 1) * BC + 1 : BC],
                            start=(ic == 0),
                            stop=(ic == JCN - 1),
                        )
                    c1 = c0 + BC  # t+1 shift (zero-padded past t=length-1)
                    acol = (tc2 * JCN + jcp) * BC + b
                    nc.vector.scalar_tensor_tensor(
                        out=scr[:, 0:tblk],
                        in0=y_ps,
                        scalar=1.0,
                        in1=oh_tiles[jcp][:, c1 : c1 + (tblk - 1) * BC + 1 : BC],
                        op0=mybir.AluOpType.mult,
                        op1=mybir.AluOpType.mult,
                        accum_out=acc2t[:, acol : acol + 1],
                    )

        # fold the per-call partial sums into numacc [128, BC]
        numacc = const.tile([128, BC], F32, name="numacc")
        rede = const.tile([128, BC], F32, name="rede")
        e_view = bass.AP(
            tensor=acc2e.tensor,
            offset=acc2e.offset,
            ap=[acc2e.ap[0], [1, BC], [BC, ne_calls]],
        )
        nc.vector.tensor_reduce(
            out=rede, in_=e_view, axis=mybir.AxisListType.X, op=mybir.AluOpType.add
        )
        t_view = bass.AP(
            tensor=acc2t.tensor,
            offset=acc2t.offset,
            ap=[acc2t.ap[0], [1, BC], [BC, nt_calls]],
        )
        redt = const.tile([128, BC], F32, name="redt")
        nc.vector.tensor_reduce(
            out=redt, in_=t_view, axis=mybir.AxisListType.X, op=mybir.AluOpType.add
        )
        nc.vector.tensor_add(out=numacc, in0=rede, in1=redt)

        ones_col_f = const.tile([128, 1], F32, name="ones_col_f")
        nc.vector.memset(ones_col_f, 1.0)

        # ---------------- finalization (per chain) ----------------
        for ch in range(NCH):
            f = fs[ch]
            fin = smallp.tile([BCH, 1], F32, name=f"fin{ch}", tag="small")
            nc.tensor.matmul(
                out=fin, lhsT=f[:, 0, :], rhs=sstop[:, 0:1], start=True, stop=False
            )
            nc.tensor.matmul(
                out=fin, lhsT=f[:, 1, :], rhs=sstop[:, 1:2], start=False, stop=True
            )
            # numerator for this chain's sequences
            numer_ps = smallp.tile([BCH, 1], F32, name=f"numer_ps{ch}", tag="small")
            nc.tensor.matmul(
                out=numer_ps,
                lhsT=numacc[:, bsl[ch]],
                rhs=ones_col_f,
                start=True,
                stop=not do_ssmm,
            )
            lastc = (length - 1) * BC
            for jc in range(JCN if do_ssmm else 0):
                nc.tensor.matmul(
                    out=numer_ps,
                    lhsT=oh_tiles[jc][:, ch * BCH : ch * BCH + BCH],
                    rhs=ssbf[:, jc : jc + 1],
                    start=False,
                    stop=False,
                )
                nc.tensor.matmul(
                    out=numer_ps,
                    lhsT=oh_tiles[jc][:, lastc + ch * BCH : lastc + ch * BCH + BCH],
                    rhs=ssbf[:, JCN + jc : JCN + jc + 1],
                    start=False,
                    stop=(jc == JCN - 1),
                )

            logtmp = const.tile([BCH, nspad], F32, name=f"logtmp{ch}")
            sumlog = const.tile([BCH, 1], F32, name=f"sumlog{ch}")
            nc.scalar.activation(
                out=logtmp,
                in_=logsbufs[ch],
                func=mybir.ActivationFunctionType.Ln,
                accum_out=sumlog,
            )
            logfin = const.tile([BCH, 1], F32, name=f"logfin{ch}")
            nc.scalar.activation(
                out=logfin, in_=fin, func=mybir.ActivationFunctionType.Ln
            )
            t3 = const.tile([BCH, 1], F32, name=f"t3{ch}")
            nc.vector.tensor_sub(out=t3, in0=numer_ps, in1=logfin)
            loss_sb = const.tile([BCH, 1], F32, name=f"loss_sb{ch}")
            # numerator used unshifted transitions; the E-side folded -CSH per
            # step: loss = t3 - CSH*(L-1) - sumlog
            nc.vector.scalar_tensor_tensor(
                out=loss_sb,
                in0=t3,
                scalar=float(CSH * (length - 1)),
                in1=sumlog,
                op0=mybir.AluOpType.subtract,
                op1=mybir.AluOpType.subtract,
            )
            nc.sync.dma_start(
                out=dram_ap(loss_t, ch * BCH, [[1, BCH], [1, 1]]), in_=loss_sb
            )

    nc.finalize()
    return nc


def host_inputs(inputs, tags, length=L):
    """Build per-core input maps (host-side sharding / layout prep only)."""
    inputs = np.asarray(inputs, dtype=np.float32)
    tags = np.asarray(tags)

    in_maps = []
    for c in range(NCORES):
        bsl = slice(c * BC, (c + 1) * BC)
        # pretranspose (layout only) to [j%128, t, j//128, b] so device loads
        # are fully contiguous per partition
        em = np.ascontiguousarray(
            inputs[bsl].reshape(BC, length, JCN, 128).transpose(3, 1, 2, 0)
        ).reshape(BC * length * T, 1)
        # tags in (t, b) order as f32 (exact for tag ids < 2^24)
        tg = np.ascontiguousarray(tags[bsl].T).astype(np.float32)
        in_maps.append(dict(em=em, tags_tb=tg.reshape(length * BC, 1)))
    return in_maps


def host_shared(transitions, start_transitions, stop_transitions):
    aux = np.zeros((AUX_N, 1), dtype=np.float32)
    # shifted by -CSH: cancels between numerator gathers and log-partition
    aux[: T * T, 0] = np.asarray(transitions, dtype=np.float32).reshape(-1) - CSH
    aux[AUX_START : AUX_START + T, 0] = np.asarray(start_transitions, np.float32)
    aux[AUX_STOP : AUX_STOP + T, 0] = np.asarray(stop_transitions, np.float32)
    aux[AUX_TRRAW :, 0] = np.asarray(transitions, dtype=np.float32).reshape(-1)
    iota = np.arange(128, dtype=np.float32).reshape(128, 1)
    return dict(aux=aux, iota=iota)


def kernel(inputs, tags, mask, transitions, start_transitions, stop_transitions):
    del mask  # all-ones per the problem spec
    in_maps = host_inputs(inputs, tags)
    shared = host_shared(transitions, start_transitions, stop_transitions)
    for m in in_maps:
        m.update(shared)

    nc = build_program()
    res = run_bass_kernel_spmd(nc, in_maps, core_ids=list(range(NCORES)))
    out = np.concatenate([r["loss"].reshape(BC) for r in res.results])
    return out.astype(np.float32)


if __name__ == "__main__":
    rng = np.random.default_rng(0)
    inputs = rng.standard_normal((B, L, T), dtype=np.float32)
    tags = rng.integers(0, T, size=(B, L))
    trans = rng.standard_normal((T, T)).astype(np.float32)
    start = rng.standard_normal(T).astype(np.float32)
    stop = rng.standard_normal(T).astype(np.float32)
    out = kernel(inputs, tags, np.ones((B, L), bool), trans, start, stop)
    print(out)


# revision 24
# speedup vs baseline: 1.0016x; 1.0016x over previous
"""ConditionalRandomField loss kernel for Trainium2 (8 NeuronCores).

Math (per sequence b):
    loss[b] = log_score(gold path) - log_partition
The log_partition forward recursion is computed in exp space:
    f_t[j] = (sum_i E[i,j] * f_{t-1}[i]) * g_t[j]
with E = exp(transitions), g_t = exp(emissions_t), f_0 = exp(start)*g_0,
and periodic per-batch rescaling whose log is accumulated separately:
    log_partition = log(sum_j f_{L-1}[j]*exp(stop[j])) + sum_m log(s_m).

Sharding: data-parallel over batch; core c owns sequences [8c, 8c+8).
Per core the 256-tag state is held as a [128 x (2 jchunk x 8 batch)] bf16
tile; each scan step is 4 PE matmuls (E tiles stationary, bf16 FWL) and one
VE multiply (psum * exp(emissions)).  The gold-path numerator uses
one-hot tag masks: emissions[b,t,tag] via tensor_tensor_reduce against the
staged emission chunks, transitions[tag_t,tag_t+1] via y = Tr^T @ OH matmuls
followed by tensor_tensor_reduce against the shifted one-hot, start/stop via
tiny matmuls.  (HW indirect-DMA only gathers one row per partition, so
per-element gathers are done with masks instead.)

NOTE: mask is all-ones for this problem spec (fill: ones); the kernel
assumes it (the reference's masked branches are identities then).
"""

import numpy as np
from contextlib import ExitStack

import concourse.bass as bass
import concourse.bacc as bacc
import concourse.tile as tile
from concourse import mybir
from concourse.bass_utils import run_bass_kernel_spmd

F32 = mybir.dt.float32


class _Bacc(bacc.Bacc):
    # Keep data waits on the MATMULs so the (data-independent) LDWEIGHTS
    # prefetch during the preceding VE phase instead of stalling the chain.
    # The PE's fg/bg weight buffers interlock LDW-vs-inflight-MM in HW.
    def move_matmul_waits_to_ldweights(self):
        super().move_matmul_waits_to_ldweights()


BF16 = mybir.dt.bfloat16
I32 = mybir.dt.int32

NCORES = 8
B = 64
L = 1024
T = 256
BC = B // NCORES      # sequences per core
PJ = 128              # partition tile of the tag dim
JCN = T // PJ         # = 2 tag chunks
RS = 16               # rescale sampling period (steps)
DEFER = 2             # rescale applied this many steps after sampling
TCH = 128             # emission-load chunk (timesteps per DMA/exp chunk)
CSH = 6.5             # constant log-shift folded into E = exp(transitions - CSH)

AUX_START = T * T          # aux table: [transitions-CSH | start | stop | transitions]
AUX_STOP = T * T + T
AUX_TRRAW = T * T + 2 * T
AUX_N = 2 * T * T + 2 * T


def _sample_steps(length):
    return [t for t in range(1, length) if t % RS == 0 and t + DEFER <= length - 1]


def build_program(length=L, use_gpsimd_oh=False, do_emis=True, do_trans=True, do_ssmm=True):
    """Build the single-core SPMD bass program (each core runs the same
    program on its own batch shard)."""
    assert length % 16 == 0
    l16 = length // 16
    nsamp = len(_sample_steps(length))
    nspad = max(8, ((nsamp + 7) // 8) * 8)

    nc = _Bacc()
    em_t = nc.declare_dram_parameter("em", [BC * length * T, 1], F32, isOutput=False)
    aux_t = nc.declare_dram_parameter("aux", [AUX_N, 1], F32, isOutput=False)
    tags_t = nc.declare_dram_parameter("tags_tb", [length * BC, 1], F32, isOutput=False)
    iota_t = nc.declare_dram_parameter("iota", [128, 1], F32, isOutput=False)
    loss_t = nc.declare_dram_parameter("loss", [BC, 1], F32, isOutput=True)

    def dram_ap(handle, offset, ap):
        full = handle[:]
        return bass.AP(tensor=full.tensor, offset=offset, ap=ap)

    with tile.TileContext(nc) as tc, ExitStack() as ctx:
        const = ctx.enter_context(tc.tile_pool(name="const", bufs=1))
        stage = ctx.enter_context(tc.tile_pool(name="stage", bufs=2))
        gpool = ctx.enter_context(tc.tile_pool(name="gpool", bufs=1))
        fpool = ctx.enter_context(tc.tile_pool(name="fpool", bufs=3))
        vpool = ctx.enter_context(tc.tile_pool(name="vpool", bufs=2))
        ppool = ctx.enter_context(tc.tile_pool(name="ppool", bufs=2, space="PSUM"))
        spool = ctx.enter_context(tc.tile_pool(name="spool", bufs=1, space="PSUM"))
        smallp = ctx.enter_context(tc.tile_pool(name="smallp", bufs=2, space="PSUM"))

        # ---------------- constants / setup ----------------
        # E = exp(transitions) as two [128, 256] bf16 tiles (i-chunk major).
        e_tiles = []
        tr_tiles = []
        for ic in range(JCN):
            eraw = stage.tile([128, T], F32, name=f"eraw{ic}", tag="eraw")
            nc.sync.dma_start(
                out=eraw,
                in_=dram_ap(aux_t, ic * 128 * T, [[T, 128], [1, T]]),
            )
            ebf = const.tile([128, T], BF16, name=f"ebf{ic}")
            # NOTE: the host uploads transitions - CSH in aux, so E here is
            # exp(transitions - CSH): per-step growth ~e^0 keeps running sums
            # inside the ACT Ln range.  The same shift flows into the
            # numerator's transition gathers, so it cancels in the loss.
            nc.scalar.activation(
                out=ebf, in_=eraw, func=mybir.ActivationFunctionType.Exp
            )
            e_tiles.append(ebf)
            # unshifted transitions in bf16 for the gold-score matmuls
            eraw2 = stage.tile([128, T], F32, name=f"eraw2_{ic}", tag="eraw")
            nc.sync.dma_start(
                out=eraw2,
                in_=dram_ap(aux_t, AUX_TRRAW + ic * 128 * T, [[T, 128], [1, T]]),
            )
            trbf = const.tile([128, T], BF16, name=f"trbf{ic}")
            nc.vector.tensor_copy(out=trbf, in_=eraw2)
            tr_tiles.append(trbf)

        # exp(start) [128, 2] f32; exp(stop) [128, 2] bf16
        ssraw = stage.tile([128, 2 * JCN], F32, name="ssraw", tag="eraw")
        nc.sync.dma_start(
            out=ssraw[:, 0:JCN],
            in_=dram_ap(aux_t, AUX_START, [[1, 128], [128, JCN]]),
        )
        nc.sync.dma_start(
            out=ssraw[:, JCN : 2 * JCN],
            in_=dram_ap(aux_t, AUX_STOP, [[1, 128], [128, JCN]]),
        )
        sstart = const.tile([128, JCN], F32, name="sstart")
        nc.scalar.activation(
            out=sstart, in_=ssraw[:, 0:JCN], func=mybir.ActivationFunctionType.Exp
        )
        sstop = const.tile([128, JCN], BF16, name="sstop")
        nc.scalar.activation(
            out=sstop, in_=ssraw[:, JCN : 2 * JCN], func=mybir.ActivationFunctionType.Exp
        )
        ssbf = const.tile([128, 2 * JCN], BF16, name="ssbf")
        nc.vector.tensor_copy(out=ssbf, in_=ssraw)

        ones_w = const.tile([128, 128], BF16, name="ones_w")
        nc.vector.memset(ones_w, 1.0)
        ones_col = const.tile([128, 1], BF16, name="ones_col")
        nc.vector.memset(ones_col, 1.0)

        logsbuf = const.tile([BC, nspad], F32, name="logsbuf")
        nc.vector.memset(logsbuf, 1.0)  # log(1)=0 padding

        # ---------------- numerator: one-hot masks ----------------
        # OH_jc[p, t*BC + b] = 1.0 iff tags[b, t] == jc*128 + p, bf16,
        # with BC zero columns of padding at t == length (for the t+1 shift).
        iota_sb = const.tile([128, 1], F32, name="iota_sb")
        nc.sync.dma_start(out=iota_sb, in_=iota_t[:])
        tags_bc = stage.tile([128, length * BC], F32, name="tags_bc", tag="tags_bc")
        nc.sync.dma_start(
            out=tags_bc,
            in_=dram_ap(tags_t, 0, [[0, 128], [1, length * BC]]),
        )
        noh = (length + 1) * BC
        oh_tiles = []
        for jc in range(JCN):
            oh = const.tile([128, noh], BF16, name=f"oh{jc}")
            oh_engine = nc.gpsimd if use_gpsimd_oh else nc.vector
            oh_engine.tensor_scalar(
                out=oh[:, 0 : length * BC],
                in0=tags_bc,
                scalar1=float(jc * 128),
                scalar2=iota_sb[:],
                op0=mybir.AluOpType.subtract,
                op1=mybir.AluOpType.is_equal,
            )
            nc.vector.memset(oh[:, length * BC : noh], 0.0)
            oh_tiles.append(oh)

        # per-(tag-partition, b) accumulators for emission+transition scores:
        # each fused multiply-accumulate call writes its partial sum into a
        # distinct column; folded at the end with two strided reduces.
        tch = min(TCH, length)
        tblk = min(512, length)
        ne_calls = (length // tch) * JCN
        nt_calls = (length // tblk) * JCN
        acc2e = const.tile([128, ne_calls * BC], F32, name="acc2e")
        acc2t = const.tile([128, nt_calls * BC], F32, name="acc2t")
        scr = const.tile([128, 512], BF16, name="scr")

        # ---------------- emissions -> g = exp(emissions), bf16 ----------------
        gbuf = gpool.tile([128, length, JCN, BC], BF16, name="gbuf")
        for tci in range(length // tch):
            raw = stage.tile([128, tch, JCN, BC], F32, name="raw", tag="raw")
            # em is host-pretransposed to [p, t, jc, b]: fully contiguous load
            row = length * JCN * BC
            nc.sync.dma_start(
                out=raw,
                in_=dram_ap(
                    em_t,
                    tci * tch * JCN * BC,
                    [[row, 128], [1, tch * JCN * BC]],
                ),
            )
            nc.scalar.activation(
                out=gbuf[:, tci * tch : (tci + 1) * tch, :, :],
                in_=raw,
                func=mybir.ActivationFunctionType.Exp,
            )
            # emission part of the gold score: sum_t raw[p,(t,jc,b)]*OH[p,t,b]
            for jc in range(JCN if do_emis else 0):
                for b in range(BC):
                    c0 = tci * tch * BC + b
                    acol = (tci * JCN + jc) * BC + b
                    nc.vector.scalar_tensor_tensor(
                        out=scr[:, 0:tch],
                        in0=raw[:, :, jc, b],
                        scalar=1.0,
                        in1=oh_tiles[jc][:, c0 : c0 + (tch - 1) * BC + 1 : BC],
                        op0=mybir.AluOpType.mult,
                        op1=mybir.AluOpType.mult,
                        accum_out=acc2e[:, acol : acol + 1],
                    )

        # ---------------- the scan: NCH independent batch-chains ----------------
        # Each chain owns BCH sequences; chains interleave on the PE so the
        # per-step latency chain (mm drain -> VE multiply -> next mm) of one
        # chain hides inside the other's.
        NCH = 2
        BCH = BC // NCH
        bsl = [slice(ch * BCH, (ch + 1) * BCH) for ch in range(NCH)]

        fs = []
        for ch in range(NCH):
            f = fpool.tile([128, JCN, BCH], BF16, name=f"f{ch}", tag=f"f{ch}")
            for jc in range(JCN):
                nc.vector.tensor_scalar_mul(
                    out=f[:, jc, :],
                    in0=gbuf[:, 0, jc, bsl[ch]],
                    scalar1=sstart[:, jc : jc + 1],
                )
            fs.append(f)

        logsbufs = []
        for ch in range(NCH):
            lsb = const.tile([BCH, nspad], F32, name=f"logsbuf{ch}")
            nc.vector.memset(lsb, 1.0)
            logsbufs.append(lsb)

        msamp = 0
        for t in range(1, length):
            sample = t % RS == 0 and t + DEFER <= length - 1
            for ch in range(NCH):
                f = fs[ch]
                p = ppool.tile([128, JCN, BCH], F32, name=f"p{ch}", tag=f"p{ch}")
                for jc in range(JCN):
                    nc.tensor.matmul(
                        out=p[:, jc, :],
                        lhsT=e_tiles[0][:, jc * 128 : (jc + 1) * 128],
                        rhs=f[:, 0, :],
                        start=True,
                        stop=False,
                    )
                    nc.tensor.matmul(
                        out=p[:, jc, :],
                        lhsT=e_tiles[1][:, jc * 128 : (jc + 1) * 128],
                        rhs=f[:, 1, :],
                        start=False,
                        stop=True,
                    )
                fn = fpool.tile([128, JCN, BCH], BF16, name=f"f{ch}", tag=f"f{ch}")
                nc.vector.tensor_tensor(
                    out=fn[:],
                    in0=p[:],
                    in1=gbuf[:, t, :, bsl[ch]],
                    op=mybir.AluOpType.mult,
                )
                fs[ch] = fn

                if sample:
                    f = fn
                    s_bc = spool.tile([128, BCH], F32, name="s_bc", tag="s")
                    nc.tensor.matmul(
                        out=s_bc, lhsT=ones_w, rhs=f[:, 0, :], start=True, stop=False
                    )
                    nc.tensor.matmul(
                        out=s_bc, lhsT=ones_w, rhs=f[:, 1, :], start=False, stop=True
                    )
                    s4 = smallp.tile([BCH, 1], F32, name="s4", tag="small")
                    nc.tensor.matmul(
                        out=s4, lhsT=f[:, 0, :], rhs=ones_col, start=True, stop=False
                    )
                    nc.tensor.matmul(
                        out=s4, lhsT=f[:, 1, :], rhs=ones_col, start=False, stop=True
                    )
                    v = vpool.tile([128, BCH], F32, name="v", tag="v")
                    nc.vector.reciprocal(out=v, in_=s_bc)
                    nc.vector.tensor_copy(
                        out=logsbufs[ch][:, msamp : msamp + 1], in_=s4
                    )
                    for jc in range(JCN):
                        nc.vector.tensor_mul(
                            out=gbuf[:, t + DEFER, jc, bsl[ch]],
                            in0=gbuf[:, t + DEFER, jc, bsl[ch]],
                            in1=v,
                        )
            if sample:
                msamp += 1
        assert msamp == nsamp

        # ---------------- transition part of the gold score ----------------
        # y[j',(t,b)] = sum_i Tr[i,j'] * OH_t[i,(t,b)]; then
        # sum_t y[j',(t,b)] * OH_{t+1}[j',(t,b)] accumulated into numacc.
        for b in range(BC if do_trans else 0):
            for tc2 in range(length // tblk):
                for jcp in range(JCN):
                    y_ps = ppool.tile([128, tblk], F32, name="y_ps", tag="p0")
                    c0 = tc2 * tblk * BC + b
                    for ic in range(JCN):
                        nc.tensor.matmul(
                            out=y_ps,
                            lhsT=tr_tiles[ic][:, jcp * 128 : (jcp + 1) * 128],
                            rhs=oh_tiles[ic][:, c0 : c0 + (tblk - 1) * BC + 1 : BC],
                            start=(ic == 0),
                            stop=(ic == JCN - 1),
                        )
                    c1 = c0 + BC  # t+1 shift (zero-padded past t=length-1)
                    acol = (tc2 * JCN + jcp) * BC + b
                    nc.vector.scalar_tensor_tensor(
                        out=scr[:, 0:tblk],
                        in0=y_ps,
                        scalar=1.0,
                        in1=oh_tiles[jcp][:, c1 : c1 + (tblk - 1) * BC + 1 : BC],
                        op0=mybir.AluOpType.mult,
                        op1=mybir.AluOpType.mult,
                        accum_out=acc2t[:, acol : acol + 1],
                    )

        # fold the per-call partial sums into numacc [128, BC]
        numacc = const.tile([128, BC], F32, name="numacc")
        rede = const.tile([128, BC], F32, name="rede")
        e_view = bass.AP(
            tensor=acc2e.tensor,
            offset=acc2e.offset,
            ap=[acc2e.ap[0], [1, BC], [BC, ne_calls]],
        )
        nc.vector.tensor_reduce(
            out=rede, in_=e_view, axis=mybir.AxisListType.X, op=mybir.AluOpType.add
        )
        t_view = bass.AP(
            tensor=acc2t.tensor,
            offset=acc2t.offset,
            ap=[acc2t.ap[0], [1, BC], [BC, nt_calls]],
        )
        redt = const.tile([128, BC], F32, name="redt")
        nc.vector.tensor_reduce(
            out=redt, in_=t_view, axis=mybir.AxisListType.X, op=mybir.AluOpType.add
        )
        nc.vector.tensor_add(out=numacc, in0=rede, in1=redt)

        ones_col_f = const.tile([128, 1], F32, name="ones_col_f")
        nc.vector.memset(ones_col_f, 1.0)

        # ---------------- finalization (per chain) ----------------
        for ch in range(NCH):
            f = fs[ch]
            fin = smallp.tile([BCH, 1], F32, name=f"fin{ch}", tag="small")
            nc.tensor.matmul(
                out=fin, lhsT=f[:, 0, :], rhs=sstop[:, 0:1], start=True, stop=False
            )
            nc.tensor.matmul(
                out=fin, lhsT=f[:, 1, :], rhs=sstop[:, 1:2], start=False, stop=True
            )
            # numerator for this chain's sequences
            numer_ps = smallp.tile([BCH, 1], F32, name=f"numer_ps{ch}", tag="small")
            nc.tensor.matmul(
                out=numer_ps,
                lhsT=numacc[:, bsl[ch]],
                rhs=ones_col_f,
                start=True,
                stop=not do_ssmm,
            )
            lastc = (length - 1) * BC
            for jc in range(JCN if do_ssmm else 0):
                nc.tensor.matmul(
                    out=numer_ps,
                    lhsT=oh_tiles[jc][:, ch * BCH : ch * BCH + BCH],
                    rhs=ssbf[:, jc : jc + 1],
                    start=False,
                    stop=False,
                )
                nc.tensor.matmul(
                    out=numer_ps,
                    lhsT=oh_tiles[jc][:, lastc + ch * BCH : lastc + ch * BCH + BCH],
                    rhs=ssbf[:, JCN + jc : JCN + jc + 1],
                    start=False,
                    stop=(jc == JCN - 1),
                )

            logtmp = const.tile([BCH, nspad], F32, name=f"logtmp{ch}")
            sumlog = const.tile([BCH, 1], F32, name=f"sumlog{ch}")
            nc.scalar.activation(
                out=logtmp,
                in_=logsbufs[ch],
                func=mybir.ActivationFunctionType.Ln,
                accum_out=sumlog,
            )
            logfin = const.tile([BCH, 1], F32, name=f"logfin{ch}")
            nc.scalar.activation(
                out=logfin, in_=fin, func=mybir.ActivationFunctionType.Ln
            )
            t3 = const.tile([BCH, 1], F32, name=f"t3{ch}")
            nc.vector.tensor_sub(out=t3, in0=numer_ps, in1=logfin)
            loss_sb = const.tile([BCH, 1], F32, name=f"loss_sb{ch}")
            # numerator used unshifted transitions; the E-side folded -CSH per
            # step: loss = t3 - CSH*(L-1) - sumlog
            nc.vector.scalar_tensor_tensor(
                out=loss_sb,
                in0=t3,
                scalar=float(CSH * (length - 1)),
                in1=sumlog,
                op0=mybir.AluOpType.subtract,
                op1=mybir.AluOpType.subtract,
            )
            nc.sync.dma_start(
                out=dram_ap(loss_t, ch * BCH, [[1, BCH], [1, 1]]), in_=loss_sb
            )

    nc.finalize()
    return nc


def host_inputs(inputs, tags, length=L):
    """Build per-core input maps (host-side sharding / layout prep only)."""
    inputs = np.asarray(inputs, dtype=np.float32)
    tags = np.asarray(tags)

    in_maps = []
    for c in range(NCORES):
        bsl = slice(c * BC, (c + 1) * BC)
        # pretranspose (layout only) to [j%128, t, j//128, b] so device loads
        # are fully contiguous per partition
        em = np.ascontiguousarray(
            inputs[bsl].reshape(BC, length, JCN, 128).transpose(3, 1, 2, 0)
        ).reshape(BC * length * T, 1)
        # tags in (t, b) order as f32 (exact for tag ids < 2^24)
        tg = np.ascontiguousarray(tags[bsl].T).astype(np.float32)
        in_maps.append(dict(em=em, tags_tb=tg.reshape(length * BC, 1)))
    return in_maps


def host_shared(transitions, start_transitions, stop_transitions):
    aux = np.zeros((AUX_N, 1), dtype=np.float32)
    # shifted by -CSH: cancels between numerator gathers and log-partition
    aux[: T * T, 0] = np.asarray(transitions, dtype=np.float32).reshape(-1) - CSH
    aux[AUX_START : AUX_START + T, 0] = np.asarray(start_transitions, np.float32)
    aux[AUX_STOP : AUX_STOP + T, 0] = np.asarray(stop_transitions, np.float32)
    aux[AUX_TRRAW :, 0] = np.asarray(transitions, dtype=np.float32).reshape(-1)
    iota = np.arange(128, dtype=np.float32).reshape(128, 1)
    return dict(aux=aux, iota=iota)


def kernel(inputs, tags, mask, transitions, start_transitions, stop_transitions):
    del mask  # all-ones per the problem spec
    in_maps = host_inputs(inputs, tags)
    shared = host_shared(transitions, start_transitions, stop_transitions)
    for m in in_maps:
        m.update(shared)

    nc = build_program()
    res = run_bass_kernel_spmd(nc, in_maps, core_ids=list(range(NCORES)))
    out = np.concatenate([r["loss"].reshape(BC) for r in res.results])
    return out.astype(np.float32)


if __name__ == "__main__":
    rng = np.random.default_rng(0)
    inputs = rng.standard_normal((B, L, T), dtype=np.float32)
    tags = rng.integers(0, T, size=(B, L))
    trans = rng.standard_normal((T, T)).astype(np.float32)
    start = rng.standard_normal(T).astype(np.float32)
    stop = rng.standard_normal(T).astype(np.float32)
    out = kernel(inputs, tags, np.ones((B, L), bool), trans, start, stop)
    print(out)
